# revision 1
# baseline (speedup 1.0000x reference)
"""Trainium2 Bass kernel for nn_Bilinear_86328842650062.

Computes out[s,i,j] = sum_{d,e} tensor1[s,i,d] * W[d,e] * tensor0[s,j,e] + bias
for S=4, N=4096, D=64, then tiles to batch 2:  output (2, 4, 4096, 4096) f32.

Strategy (classic 1D row-parallel): shard the i axis (rows of tensor1 /
rows of the output) across 8 NeuronCores, 512 rows each; replicate the
small (D,D) kernel and tensor0.  Per core and per s:
    B[s] = x1_shard[s] @ W            (512x64 @ 64x64, one matmul)
    out_shard[s] = B[s] @ x0[s]^T     (512x64 @ 64x4096, 4x8 PE tiles)
Host-side we pre-transpose x0/x1 so the contraction dim (64) lands on
SBUF partitions, packing two s-slices per 128 partitions for
full-bandwidth DMA.  The batch-2 leading dim is a pure broadcast and is
materialized host-side as a stride-0 view.
"""

import numpy as np

S, N, D = 4, 4096, 64
N_CORES = 8
ROWS = N // N_CORES  # 512 output rows per core
BATCH = 2

MM_DT = "float32"  # matmul dtype: "float32" (exact) or "float32r" (fast)

_CACHE = {}


def _build(mm_dt_name):
    import concourse.bacc as bacc
    import concourse.tile as tile
    import concourse.mybir as mybir

    dt = mybir.dt
    f32 = dt.float32
    mm_dt = getattr(dt, mm_dt_name)

    nc = bacc.Bacc(
        "TRN2",
        target_bir_lowering=False,
        debug=False,
        enable_asserts=False,
        num_devices=N_CORES,
    )
    # DRAM I/O. x1t is tensor1 shard transposed to (S, D, ROWS); x0t is
    # tensor0 transposed to (S, D, N). Both get loaded with two s-slices
    # packed per 128 SBUF partitions: partition p = 64*(s%2) + d.
    w_dram = nc.dram_tensor("w", [D, D], f32, kind="ExternalInput").ap()
    x1t_dram = nc.dram_tensor("x1t", [S, D, ROWS], f32, kind="ExternalInput").ap()
    x0t_dram = nc.dram_tensor("x0t", [S, D, N], f32, kind="ExternalInput").ap()
    out_dram = nc.dram_tensor("out", [S, ROWS, N], f32, kind="ExternalOutput").ap()

    IT = ROWS // 128  # 4 psum row-tiles per s
    JT = N // 512     # 8 psum col-tiles per row-tile

    with tile.TileContext(nc) as tc:
        with (
            tc.tile_pool(name="const", bufs=1) as const_pool,
            tc.tile_pool(name="bt", bufs=2) as bt_pool,
            tc.tile_pool(name="outsb", bufs=3) as out_pool,
            tc.tile_pool(name="psb", bufs=2, space="PSUM") as psb_pool,
            tc.tile_pool(name="pso", bufs=4, space="PSUM") as pso_pool,
        ):
            # W replicated to both partition halves so lhsT/rhs base
            # partitions match for odd s.
            wt = const_pool.tile([128, D], f32)
            nc.sync.dma_start(wt[0:D, :], w_dram[:])
            nc.sync.dma_start(wt[D : 2 * D, :], w_dram[:])

            # (S, D, X) -> sbuf [128, S//2, X]: partition p = 64*(s%2)+d,
            # free a = s//2.  In DRAM, (s, d) flattens to p-major order
            # (a p) since stride(s) = D*X and stride(d) = X.
            x1t_sb = const_pool.tile([128, S // 2, ROWS], f32)
            x0t_sb = const_pool.tile([128, S // 2, N], f32)
            x1_r = x1t_dram.rearrange("(a ps) d x -> (ps d) a x", ps=2)
            x0_r = x0t_dram.rearrange("(a ps) d x -> (ps d) a x", ps=2)
            nc.sync.dma_start(x1t_sb[:], x1_r)
            for a in range(S // 2):
                nc.sync.dma_start(x0t_sb[:, a, :], x0_r[:, a, :])

            for s in range(S):
                p0 = (s % 2) * D
                a = s // 2
                # B^T[s] = (x1[s] @ W)^T : psum[e, i] = sum_d W[d,e] x1t[d,i]
                ps_b = psb_pool.tile([D, ROWS], f32)
                nc.tensor.matmul(
                    ps_b[:],
                    wt[p0 : p0 + D, :].bitcast(mm_dt),
                    x1t_sb[p0 : p0 + D, a, :].bitcast(mm_dt),
                    start=True,
                    stop=True,
                )
                bt = bt_pool.tile([128, ROWS], f32)
                nc.vector.tensor_copy(bt[p0 : p0 + D, :], ps_b[:])

                for it in range(IT):
                    out_sb = out_pool.tile([128, N], f32)
                    for jt in range(JT):
                        ps_o = pso_pool.tile([128, 512], f32)
                        # out[i, j] = sum_e B^T[e,i] x0t[e,j]
                        nc.tensor.matmul(
                            ps_o[:],
                            bt[p0 : p0 + D, it * 128 : (it + 1) * 128].bitcast(mm_dt),
                            x0t_sb[p0 : p0 + D, a, jt * 512 : (jt + 1) * 512].bitcast(
                                mm_dt
                            ),
                            start=True,
                            stop=True,
                        )
                        nc.vector.tensor_copy(out_sb[:, jt * 512 : (jt + 1) * 512], ps_o[:])
                    nc.sync.dma_start(
                        out_dram[s, it * 128 : (it + 1) * 128, :], out_sb[:]
                    )
    nc.compile()
    return nc


def _get_nc():
    if MM_DT not in _CACHE:
        _CACHE[MM_DT] = _build(MM_DT)
    return _CACHE[MM_DT]


LAST_RESULTS = None


def kernel(**inputs):
    from concourse.bass_utils import run_bass_kernel_spmd

    global LAST_RESULTS

    tensor0 = np.ascontiguousarray(np.asarray(inputs["tensor0"], dtype=np.float32))
    tensor1 = np.ascontiguousarray(np.asarray(inputs["tensor1"], dtype=np.float32))
    W = np.ascontiguousarray(np.asarray(inputs["kernel"], dtype=np.float32))
    bias = float(np.asarray(inputs["bias"]))

    # Host prep: put the contraction dim on axis -2 for partition-major DMA.
    x0t = np.ascontiguousarray(tensor0.transpose(0, 2, 1))  # (S, D, N)
    x1t_full = tensor1.transpose(0, 2, 1)  # (S, D, N) view

    in_maps = []
    for c in range(N_CORES):
        in_maps.append(
            {
                "w": W,
                "x1t": np.ascontiguousarray(
                    x1t_full[:, :, c * ROWS : (c + 1) * ROWS]
                ),
                "x0t": x0t,
            }
        )

    nc = _get_nc()
    res = run_bass_kernel_spmd(nc, in_maps, list(range(N_CORES)))
    LAST_RESULTS = res

    out_full = np.empty((S, N, N), dtype=np.float32)
    for c in range(N_CORES):
        out_full[:, c * ROWS : (c + 1) * ROWS, :] = res.results[c]["out"]

    if bias != 0.0:
        out_full += np.float32(bias)

    return np.broadcast_to(out_full[None], (BATCH, S, N, N))


# revision 6
# speedup vs baseline: 1.3439x; 1.3439x over previous
"""Trainium2 Bass kernel for nn_Bilinear_86328842650062.

Computes out[s,i,j] = sum_{d,e} tensor1[s,i,d] * W[d,e] * tensor0[s,j,e] + bias
for S=4, N=4096, D=64, then tiles to batch 2:  output (2, 4, 4096, 4096) f32.

Strategy (classic 1D row-parallel): shard the i axis (rows of tensor1 /
rows of the output) across 8 NeuronCores, 512 rows each; replicate the
small (D,D) kernel and tensor0.  Per core and per s:
    B[s] = x1_shard[s] @ W            (512x64 @ 64x64, one matmul)
    out_shard[s] = B[s] @ x0[s]^T     (512x64 @ 64x4096, 4x8 PE tiles)
Host-side we pre-transpose x0/x1 so the contraction dim (64) lands on
SBUF partitions, packing two s-slices per 128 partitions for
full-bandwidth DMA.  The batch-2 leading dim is a pure broadcast and is
materialized host-side as a stride-0 view.
"""

import numpy as np

S, N, D = 4, 4096, 64
N_CORES = 8
ROWS = N // N_CORES  # 512 output rows per core
BATCH = 2

import os as _os

MM_DT = _os.environ.get("BASS_MM_DT", "float32")  # "float32" (exact) or "float32r" (fast)

_CACHE = {}


def _build(mm_dt_name):
    import concourse.bacc as bacc
    import concourse.tile as tile
    import concourse.mybir as mybir

    dt = mybir.dt
    f32 = dt.float32
    mm_dt = getattr(dt, mm_dt_name)

    nc = bacc.Bacc(
        "TRN2",
        target_bir_lowering=False,
        debug=False,
        enable_asserts=False,
        num_devices=N_CORES,
    )
    # DRAM I/O. x1t is tensor1 shard transposed to (S, D, ROWS); x0t is
    # tensor0 transposed to (S, D, N). Both get loaded with two s-slices
    # packed per 128 SBUF partitions: partition p = 64*(s%2) + d.
    w_dram = nc.dram_tensor("w", [D, D], mm_dt, kind="ExternalInput").ap()
    x1t_dram = nc.dram_tensor("x1t", [S, D, ROWS], mm_dt, kind="ExternalInput").ap()
    x0t_dram = nc.dram_tensor("x0t", [S, D, N], mm_dt, kind="ExternalInput").ap()
    out_dram = nc.dram_tensor("out", [S, ROWS, N], f32, kind="ExternalOutput").ap()

    IT = ROWS // 128  # 4 psum row-tiles per s
    JT = N // 512     # 8 psum col-tiles per row-tile

    with tile.TileContext(nc) as tc:
        with (
            tc.tile_pool(name="const", bufs=1) as const_pool,
            tc.tile_pool(name="bt", bufs=2) as bt_pool,
            tc.tile_pool(name="outsb", bufs=3) as out_pool,
            tc.tile_pool(name="psb", bufs=2, space="PSUM") as psb_pool,
            tc.tile_pool(name="pso", bufs=4, space="PSUM") as pso_pool,
        ):
            # W replicated to both partition halves so lhsT/rhs base
            # partitions match for odd s.
            wt = const_pool.tile([128, D], mm_dt)
            nc.sync.dma_start(wt[0:D, :], w_dram[:])
            nc.sync.dma_start(wt[D : 2 * D, :], w_dram[:])

            # (S, D, X) -> sbuf [128, S//2, X]: partition p = 64*(s%2)+d,
            # free a = s//2.  In DRAM, (s, d) flattens to p-major order
            # (a p) since stride(s) = D*X and stride(d) = X.
            x1t_sb = const_pool.tile([128, S // 2, ROWS], mm_dt)
            x0t_sb = const_pool.tile([128, S // 2, N], mm_dt)
            x1_r = x1t_dram.rearrange("(a ps) d x -> (ps d) a x", ps=2)
            x0_r = x0t_dram.rearrange("(a ps) d x -> (ps d) a x", ps=2)
            nc.sync.dma_start(x1t_sb[:], x1_r)
            for a in range(S // 2):
                nc.sync.dma_start(x0t_sb[:, a, :], x0_r[:, a, :])

            for s in range(S):
                p0 = (s % 2) * D
                a = s // 2
                # B^T[s] = (x1[s] @ W)^T : psum[e, i] = sum_d W[d,e] x1t[d,i]
                ps_b = psb_pool.tile([D, ROWS], f32)
                nc.tensor.matmul(
                    ps_b[:],
                    wt[p0 : p0 + D, :],
                    x1t_sb[p0 : p0 + D, a, :],
                    start=True,
                    stop=True,
                )
                bt = bt_pool.tile([128, ROWS], mm_dt)
                nc.vector.tensor_copy(bt[p0 : p0 + D, :], ps_b[:])

                for it in range(IT):
                    out_sb = out_pool.tile([128, N], f32)
                    for jt in range(JT):
                        ps_o = pso_pool.tile([128, 512], f32)
                        # out[i, j] = sum_e B^T[e,i] x0t[e,j]
                        nc.tensor.matmul(
                            ps_o[:],
                            bt[p0 : p0 + D, it * 128 : (it + 1) * 128],
                            x0t_sb[p0 : p0 + D, a, jt * 512 : (jt + 1) * 512],
                            start=True,
                            stop=True,
                        )
                        nc.vector.tensor_copy(out_sb[:, jt * 512 : (jt + 1) * 512], ps_o[:])
                    nc.sync.dma_start(
                        out_dram[s, it * 128 : (it + 1) * 128, :], out_sb[:]
                    )
    nc.compile()
    return nc


def _get_nc():
    if MM_DT not in _CACHE:
        _CACHE[MM_DT] = _build(MM_DT)
    return _CACHE[MM_DT]


LAST_RESULTS = None


def kernel(**inputs):
    from concourse.bass_utils import run_bass_kernel_spmd

    global LAST_RESULTS

    tensor0 = np.ascontiguousarray(np.asarray(inputs["tensor0"], dtype=np.float32))
    tensor1 = np.ascontiguousarray(np.asarray(inputs["tensor1"], dtype=np.float32))
    W = np.ascontiguousarray(np.asarray(inputs["kernel"], dtype=np.float32))
    bias = float(np.asarray(inputs["bias"]))

    # Host prep: put the contraction dim on axis -2 for partition-major DMA.
    x0t = np.ascontiguousarray(tensor0.transpose(0, 2, 1))  # (S, D, N)
    x1t_full = tensor1.transpose(0, 2, 1)  # (S, D, N) view

    in_maps = []
    for c in range(N_CORES):
        in_maps.append(
            {
                "w": W,
                "x1t": np.ascontiguousarray(
                    x1t_full[:, :, c * ROWS : (c + 1) * ROWS]
                ),
                "x0t": x0t,
            }
        )

    nc = _get_nc()
    res = run_bass_kernel_spmd(nc, in_maps, list(range(N_CORES)))
    LAST_RESULTS = res

    out_full = np.empty((S, N, N), dtype=np.float32)
    for c in range(N_CORES):
        out_full[:, c * ROWS : (c + 1) * ROWS, :] = res.results[c]["out"]

    if bias != 0.0:
        out_full += np.float32(bias)

    return np.broadcast_to(out_full[None], (BATCH, S, N, N))


# revision 8
# speedup vs baseline: 1.3531x; 1.0068x over previous
"""Trainium2 Bass kernel for nn_Bilinear_86328842650062.

Computes out[s,i,j] = sum_{d,e} tensor1[s,i,d] * W[d,e] * tensor0[s,j,e] + bias
for S=4, N=4096, D=64, then tiles to batch 2:  output (2, 4, 4096, 4096) f32.

Strategy (classic 1D row-parallel): shard the i axis (rows of tensor1 /
rows of the output) across 8 NeuronCores, 512 rows each; replicate the
small (D,D) kernel and tensor0.  Per core and per s:
    B[s] = x1_shard[s] @ W            (512x64 @ 64x64, one matmul)
    out_shard[s] = B[s] @ x0[s]^T     (512x64 @ 64x4096, 4x8 PE tiles)
Host-side we pre-transpose x0/x1 so the contraction dim (64) lands on
SBUF partitions, packing two s-slices per 128 partitions for
full-bandwidth DMA.  The batch-2 leading dim is a pure broadcast and is
materialized host-side as a stride-0 view.
"""

import numpy as np

S, N, D = 4, 4096, 64
N_CORES = 8
ROWS = N // N_CORES  # 512 output rows per core
BATCH = 2

import os as _os

MM_DT = _os.environ.get("BASS_MM_DT", "float32")  # "float32" (exact) or "float32r" (fast)

_CACHE = {}


def _build(mm_dt_name):
    import concourse.bacc as bacc
    import concourse.tile as tile
    import concourse.mybir as mybir

    dt = mybir.dt
    f32 = dt.float32
    mm_dt = getattr(dt, mm_dt_name)

    nc = bacc.Bacc(
        "TRN2",
        target_bir_lowering=False,
        debug=False,
        enable_asserts=False,
        num_devices=N_CORES,
    )
    # DRAM I/O. x1t is tensor1 shard transposed to (S, D, ROWS); x0t is
    # tensor0 transposed to (S, D, N). Both get loaded with two s-slices
    # packed per 128 SBUF partitions: partition p = 64*(s%2) + d.
    w_dram = nc.dram_tensor("w", [D, D], mm_dt, kind="ExternalInput").ap()
    x1t_dram = nc.dram_tensor("x1t", [S, D, ROWS], mm_dt, kind="ExternalInput").ap()
    x0t_dram = nc.dram_tensor("x0t", [S, D, N], mm_dt, kind="ExternalInput").ap()
    out_dram = nc.dram_tensor("out", [S, ROWS, N], f32, kind="ExternalOutput").ap()

    IT = ROWS // 128  # 4 psum row-tiles per s
    JT = N // 512     # 8 psum col-tiles per row-tile

    with tile.TileContext(nc) as tc:
        with (
            tc.tile_pool(name="const", bufs=1) as const_pool,
            tc.tile_pool(name="bt", bufs=2) as bt_pool,
            tc.tile_pool(name="outsb", bufs=3) as out_pool,
            tc.tile_pool(name="psb", bufs=2, space="PSUM") as psb_pool,
            tc.tile_pool(name="pso", bufs=3, space="PSUM") as pso_pool,
        ):
            # W replicated to both partition halves so lhsT/rhs base
            # partitions match for odd s.
            wt = const_pool.tile([128, D], mm_dt)
            nc.sync.dma_start(wt[0:D, :], w_dram[:])
            nc.sync.dma_start(wt[D : 2 * D, :], w_dram[:])

            # (S, D, X) -> sbuf [128, S//2, X]: partition p = 64*(s%2)+d,
            # free a = s//2.  In DRAM, (s, d) flattens to p-major order
            # (a p) since stride(s) = D*X and stride(d) = X.
            x1t_sb = const_pool.tile([128, S // 2, ROWS], mm_dt)
            x0t_sb = const_pool.tile([128, S // 2, N], mm_dt)
            x1_r = x1t_dram.rearrange("(a ps) d x -> (ps d) a x", ps=2)
            x0_r = x0t_dram.rearrange("(a ps) d x -> (ps d) a x", ps=2)
            nc.sync.dma_start(x1t_sb[:], x1_r)
            for a in range(S // 2):
                nc.sync.dma_start(x0t_sb[:, a, :], x0_r[:, a, :])

            for s in range(S):
                p0 = (s % 2) * D
                a = s // 2
                # B^T[s] = (x1[s] @ W)^T : psum[e, i] = sum_d W[d,e] x1t[d,i]
                ps_b = psb_pool.tile([D, ROWS], f32)
                nc.tensor.matmul(
                    ps_b[:],
                    wt[p0 : p0 + D, :],
                    x1t_sb[p0 : p0 + D, a, :],
                    start=True,
                    stop=True,
                )
                bt = bt_pool.tile([128, ROWS], mm_dt)
                nc.vector.tensor_copy(bt[p0 : p0 + D, :], ps_b[:])

                for it in range(IT):
                    out_sb = out_pool.tile([128, N], f32)
                    # Pair two matmuls into a 2-bank psum tile so each
                    # copy-back moves 1024 cols (amortizes per-op overhead);
                    # route 1/4 of the copies to the otherwise-idle ACT.
                    for jt2 in range(JT // 2):
                        ps_o = pso_pool.tile([128, 1024], f32)
                        for h in range(2):
                            jt = jt2 * 2 + h
                            # out[i, j] = sum_e B^T[e,i] x0t[e,j]
                            nc.tensor.matmul(
                                ps_o[:, h * 512 : (h + 1) * 512],
                                bt[p0 : p0 + D, it * 128 : (it + 1) * 128],
                                x0t_sb[p0 : p0 + D, a, jt * 512 : (jt + 1) * 512],
                                start=True,
                                stop=True,
                            )
                        dst = out_sb[:, jt2 * 1024 : (jt2 + 1) * 1024]
                        if jt2 == JT // 2 - 1:
                            nc.scalar.copy(dst, ps_o[:])
                        else:
                            nc.vector.tensor_copy(dst, ps_o[:])
                    nc.sync.dma_start(
                        out_dram[s, it * 128 : (it + 1) * 128, :], out_sb[:]
                    )
    nc.compile()
    return nc


def _get_nc():
    if MM_DT not in _CACHE:
        _CACHE[MM_DT] = _build(MM_DT)
    return _CACHE[MM_DT]


LAST_RESULTS = None


def kernel(**inputs):
    from concourse.bass_utils import run_bass_kernel_spmd

    global LAST_RESULTS

    tensor0 = np.ascontiguousarray(np.asarray(inputs["tensor0"], dtype=np.float32))
    tensor1 = np.ascontiguousarray(np.asarray(inputs["tensor1"], dtype=np.float32))
    W = np.ascontiguousarray(np.asarray(inputs["kernel"], dtype=np.float32))
    bias = float(np.asarray(inputs["bias"]))

    # Host prep: put the contraction dim on axis -2 for partition-major DMA.
    x0t = np.ascontiguousarray(tensor0.transpose(0, 2, 1))  # (S, D, N)
    x1t_full = tensor1.transpose(0, 2, 1)  # (S, D, N) view

    in_maps = []
    for c in range(N_CORES):
        in_maps.append(
            {
                "w": W,
                "x1t": np.ascontiguousarray(
                    x1t_full[:, :, c * ROWS : (c + 1) * ROWS]
                ),
                "x0t": x0t,
            }
        )

    nc = _get_nc()
    res = run_bass_kernel_spmd(nc, in_maps, list(range(N_CORES)))
    LAST_RESULTS = res

    out_full = np.empty((S, N, N), dtype=np.float32)
    for c in range(N_CORES):
        out_full[:, c * ROWS : (c + 1) * ROWS, :] = res.results[c]["out"]

    if bias != 0.0:
        out_full += np.float32(bias)

    return np.broadcast_to(out_full[None], (BATCH, S, N, N))


# revision 11
# speedup vs baseline: 1.5568x; 1.1505x over previous
"""Trainium2 Bass kernel for nn_Bilinear_86328842650062.

Computes out[s,i,j] = sum_{d,e} tensor1[s,i,d] * W[d,e] * tensor0[s,j,e] + bias
for S=4, N=4096, D=64, then tiles to batch 2:  output (2, 4, 4096, 4096) f32.

Strategy (classic 1D row-parallel): shard the i axis (rows of tensor1 /
rows of the output) across 8 NeuronCores, 512 rows each; replicate the
small (D,D) kernel and tensor0.  Per core and per s:
    B[s] = x1_shard[s] @ W            (512x64 @ 64x64, one matmul)
    out_shard[s] = B[s] @ x0[s]^T     (512x64 @ 64x4096, 4x8 PE tiles)
Host-side we pre-transpose x0/x1 so the contraction dim (64) lands on
SBUF partitions, packing two s-slices per 128 partitions for
full-bandwidth DMA.  The batch-2 leading dim is a pure broadcast and is
materialized host-side as a stride-0 view.
"""

import numpy as np

S, N, D = 4, 4096, 64
N_CORES = 8
ROWS = N // N_CORES  # 512 output rows per core
BATCH = 2

import os as _os

MM_DT = _os.environ.get("BASS_MM_DT", "float32")  # "float32" (exact) or "float32r" (fast)

_CACHE = {}


def _build(mm_dt_name):
    import concourse.bacc as bacc
    import concourse.tile as tile
    import concourse.mybir as mybir

    dt = mybir.dt
    f32 = dt.float32
    mm_dt = getattr(dt, mm_dt_name)

    nc = bacc.Bacc(
        "TRN2",
        target_bir_lowering=False,
        debug=False,
        enable_asserts=False,
        num_devices=N_CORES,
    )
    # DRAM I/O. x1t is tensor1 shard transposed to (S, D, ROWS); x0t is
    # tensor0 transposed to (S, D, N). Both get loaded with two s-slices
    # packed per 128 SBUF partitions: partition p = 64*(s%2) + d.
    w_dram = nc.dram_tensor("w", [D, D], mm_dt, kind="ExternalInput").ap()
    x1t_dram = nc.dram_tensor("x1t", [S, D, ROWS], mm_dt, kind="ExternalInput").ap()
    x0t_dram = nc.dram_tensor("x0t", [S, D, N], mm_dt, kind="ExternalInput").ap()
    out_dram = nc.dram_tensor("out", [S, ROWS, N], f32, kind="ExternalOutput").ap()

    IT = ROWS // 128  # 4 psum row-tiles per s
    JT = N // 512     # 8 psum col-tiles per row-tile

    with tile.TileContext(nc) as tc:
        with (
            tc.tile_pool(name="const", bufs=1) as const_pool,
            tc.tile_pool(name="bt", bufs=2) as bt_pool,
            tc.tile_pool(name="outsb", bufs=3) as out_pool,
            tc.tile_pool(name="psb", bufs=2, space="PSUM") as psb_pool,
            tc.tile_pool(name="pso", bufs=3, space="PSUM") as pso_pool,
        ):
            # W replicated to both partition halves so lhsT/rhs base
            # partitions match for odd s.
            wt = const_pool.tile([128, D], mm_dt)
            nc.gpsimd.dma_start(wt[0:D, :], w_dram[:])
            nc.gpsimd.dma_start(wt[D : 2 * D, :], w_dram[:])

            # (S, D, X) -> sbuf [128, S//2, X]: partition p = 64*(s%2)+d,
            # free a = s//2.  In DRAM, (s, d) flattens to p-major order
            # (a p) since stride(s) = D*X and stride(d) = X.
            # Inputs ride the gpsimd (SWDGE) ring so the sync/scalar HWDGE
            # rings stay clear for output; x0t lands in 1 MiB column chunks
            # so the first matmuls start as early as possible.
            x1t_sb = const_pool.tile([128, S // 2, ROWS], mm_dt)
            x0t_sb = const_pool.tile([128, S // 2, N], mm_dt)
            x1_r = x1t_dram.rearrange("(a ps) d x -> (ps d) a x", ps=2)
            x0_r = x0t_dram.rearrange("(a ps) d x -> (ps d) a x", ps=2)
            nc.gpsimd.dma_start(x1t_sb[:], x1_r)
            for a in range(S // 2):
                for jh in range(2):
                    nc.gpsimd.dma_start(
                        x0t_sb[:, a, jh * (N // 2) : (jh + 1) * (N // 2)],
                        x0_r[:, a, jh * (N // 2) : (jh + 1) * (N // 2)],
                    )

            for s in range(S):
                p0 = (s % 2) * D
                a = s // 2
                # B^T[s] = (x1[s] @ W)^T : psum[e, i] = sum_d W[d,e] x1t[d,i]
                ps_b = psb_pool.tile([D, ROWS], f32)
                nc.tensor.matmul(
                    ps_b[:],
                    wt[p0 : p0 + D, :],
                    x1t_sb[p0 : p0 + D, a, :],
                    start=True,
                    stop=True,
                )
                bt = bt_pool.tile([128, ROWS], mm_dt)
                nc.vector.tensor_copy(bt[p0 : p0 + D, :], ps_b[:])

                for it in range(IT):
                    out_sb = out_pool.tile([128, N], f32)
                    # Pair two matmuls into a 2-bank psum tile so each
                    # copy-back moves 1024 cols (amortizes per-op overhead);
                    # route 1/4 of the copies to the otherwise-idle ACT.
                    for jt2 in range(JT // 2):
                        ps_o = pso_pool.tile([128, 1024], f32)
                        for h in range(2):
                            jt = jt2 * 2 + h
                            # out[i, j] = sum_e B^T[e,i] x0t[e,j]
                            nc.tensor.matmul(
                                ps_o[:, h * 512 : (h + 1) * 512],
                                bt[p0 : p0 + D, it * 128 : (it + 1) * 128],
                                x0t_sb[p0 : p0 + D, a, jt * 512 : (jt + 1) * 512],
                                start=True,
                                stop=True,
                            )
                        dst = out_sb[:, jt2 * 1024 : (jt2 + 1) * 1024]
                        nc.vector.tensor_copy(dst, ps_o[:])
                        if jt2 % 2 == 1:
                            # Drain each finished 2048-col half-block right
                            # away (1 MiB DMA), alternating HWDGE rings.
                            jh = jt2 // 2
                            eng = nc.sync if (it + jh) % 2 == 0 else nc.scalar
                            eng.dma_start(
                                out_dram[
                                    s,
                                    it * 128 : (it + 1) * 128,
                                    jh * 2048 : (jh + 1) * 2048,
                                ],
                                out_sb[:, jh * 2048 : (jh + 1) * 2048],
                            )
    nc.compile()
    return nc


def _get_nc():
    if MM_DT not in _CACHE:
        _CACHE[MM_DT] = _build(MM_DT)
    return _CACHE[MM_DT]


LAST_RESULTS = None


def kernel(**inputs):
    from concourse.bass_utils import run_bass_kernel_spmd

    global LAST_RESULTS

    tensor0 = np.ascontiguousarray(np.asarray(inputs["tensor0"], dtype=np.float32))
    tensor1 = np.ascontiguousarray(np.asarray(inputs["tensor1"], dtype=np.float32))
    W = np.ascontiguousarray(np.asarray(inputs["kernel"], dtype=np.float32))
    bias = float(np.asarray(inputs["bias"]))

    # Host prep: put the contraction dim on axis -2 for partition-major DMA.
    x0t = np.ascontiguousarray(tensor0.transpose(0, 2, 1))  # (S, D, N)
    x1t_full = tensor1.transpose(0, 2, 1)  # (S, D, N) view

    in_maps = []
    for c in range(N_CORES):
        in_maps.append(
            {
                "w": W,
                "x1t": np.ascontiguousarray(
                    x1t_full[:, :, c * ROWS : (c + 1) * ROWS]
                ),
                "x0t": x0t,
            }
        )

    nc = _get_nc()
    res = run_bass_kernel_spmd(nc, in_maps, list(range(N_CORES)))
    LAST_RESULTS = res

    out_full = np.empty((S, N, N), dtype=np.float32)
    for c in range(N_CORES):
        out_full[:, c * ROWS : (c + 1) * ROWS, :] = res.results[c]["out"]

    if bias != 0.0:
        out_full += np.float32(bias)

    return np.broadcast_to(out_full[None], (BATCH, S, N, N))
